# revision 8
# baseline (speedup 1.0000x reference)
"""Bayes-by-backprop linear layer on 8 Trainium2 NeuronCores.

Reference computation (per MC sample s of S=8):
    W_s  = weight_mu + softplus(weight_rho) * eps_w[s]     (2048 x 2048)
    b_s  = bias_mu   + softplus(bias_rho)   * eps_b[s]     (2048,)
    y[s] = x[s] @ W_s.T + b_s                              (256 x 2048)

eps_w / eps_b come from jax.random with the fixed key 42 — they are
input-independent constants, so they are generated host-side once (JAX
threefry is bitwise deterministic across backends) and shipped to the
device; all input-dependent math (softplus, W construction, GEMM, bias)
runs on the NeuronCores.

Sharding (8 cores): 2 sample-groups (4 samples) x 4 out-column slices
(512 cols).  This minimizes HBM traffic vs pure sample sharding: eps is
split exactly 8 ways either way, but mu/rho are loaded 4x (not 8x) and
x only 2x.

Device layout: everything is pre-transposed on host so the contraction
dim i (IN) lands on SBUF partitions:
    xT  [s, p, ko, b]  = x[s, b, ko*128+p]        (stationary operand)
    muT/rhoT/epsT [p, ko, o] = M[o, ko*128+p]     (moving operand source)
PSUM out tile [b_part(128), o(512)] = xT_tile.T @ WT_tile, accumulated
over ko = 0..15; so y comes back already in (b, o) layout.
"""

import numpy as np

S, B, IN, OUT = 8, 256, 2048, 2048
NCORES = 8
GS, JS = 2, 4              # sample-group shards x out-column shards
S_LOC = S // GS            # 4 samples per core
O_LOC = OUT // JS          # 512 out cols per core
P = 128
KO = IN // P               # 16 k-tiles
B_TILES = B // P           # 2

# matmul operand dtype: float32r streams at 1 elem/cycle (bf16 speed) with
# fp32 operand bits; plain float32 is 4x slower. Set via module flag so the
# test harness can A/B them.
MM_DT = "float32r"
TRACE = False
LAST_RESULT = None

_cache = {}


def _get_eps():
    """Constant eps tensors (key 42), in device layouts, cached per core."""
    if "eps_cores" in _cache:
        return _cache["eps_cores"], _cache["eps_b"]
    import jax
    import jax.numpy as jnp

    cpu = jax.devices("cpu")[0]
    with jax.default_device(cpu):
        kw, kb = jax.random.split(jax.random.key(42))
        eps_w = np.asarray(jax.random.normal(kw, (S, OUT, IN), dtype=jnp.float32))
        eps_b = np.asarray(jax.random.normal(kb, (S, OUT), dtype=jnp.float32))
    # epsT[s, p, ko, o] = eps_w[s, o, ko*128+p]
    epsT = eps_w.reshape(S, OUT, KO, P).transpose(0, 3, 2, 1)
    eps_cores = {}
    for c in range(NCORES):
        g, j = divmod(c, JS)
        eps_cores[c] = np.ascontiguousarray(
            epsT[g * S_LOC:(g + 1) * S_LOC, :, :, j * O_LOC:(j + 1) * O_LOC]
        )
    _cache["eps_cores"] = eps_cores
    _cache["eps_b"] = eps_b
    return eps_cores, eps_b


def _build_nc(mm_dt_name):
    key = ("nc", mm_dt_name)
    if key in _cache:
        return _cache[key]
    import concourse.mybir as mybir
    import concourse.tile as tile
    from concourse import bacc

    f32 = mybir.dt.float32
    mm_dt = getattr(mybir.dt, mm_dt_name)
    EXP = mybir.ActivationFunctionType.Exp
    LN = mybir.ActivationFunctionType.Ln

    nc = bacc.Bacc(trn_type="TRN2")
    xT = nc.dram_tensor("xT", (S_LOC, P, KO, B), mm_dt, kind="ExternalInput")
    muT = nc.dram_tensor("muT", (P, KO, O_LOC), f32, kind="ExternalInput")
    rhoT = nc.dram_tensor("rhoT", (P, KO, O_LOC), f32, kind="ExternalInput")
    epsT = nc.dram_tensor("epsT", (S_LOC, P, KO, O_LOC), f32, kind="ExternalInput")
    bmu = nc.dram_tensor("bmu", (P, O_LOC), f32, kind="ExternalInput")
    brho = nc.dram_tensor("brho", (P, O_LOC), f32, kind="ExternalInput")
    beps = nc.dram_tensor("beps", (P, S_LOC, O_LOC), f32, kind="ExternalInput")
    y = nc.dram_tensor("y", (S_LOC, B_TILES, P, O_LOC), f32, kind="ExternalOutput")

    # fp32r matmul operands must be *written* as float32r by their producer
    # (BIR verifier checkMatmultFP32r), so the operand tiles themselves carry
    # mm_dt and the producers (DMA for xT, DVE for w) write that dtype.

    with tile.TileContext(nc) as tc:
        with (
            tc.tile_pool(name="const", bufs=1) as cpool,
            tc.tile_pool(name="eps", bufs=2) as epool,
            tc.tile_pool(name="w", bufs=4) as wpool,
            tc.tile_pool(name="out", bufs=2) as opool,
            tc.tile_pool(name="psum", bufs=4, space="PSUM") as ppool,
        ):
            # rho first (softplus is on the critical path for the first W)
            sig_sb = cpool.tile([P, KO, O_LOC], f32)
            nc.sync.dma_start(sig_sb[:], rhoT[:])
            xt_sb = cpool.tile([P, S_LOC, KO, B], mm_dt)
            nc.sync.dma_start(xt_sb[:, 0], xT[0])
            mu_sb = cpool.tile([P, KO, O_LOC], f32)
            nc.sync.dma_start(mu_sb[:], muT[:])
            nc.scalar.activation(sig_sb[:], sig_sb[:], EXP)
            nc.scalar.activation(sig_sb[:], sig_sb[:], LN, bias=1.0)

            # bias: b_s = bmu + softplus(brho) * beps[s], all pre-replicated
            # across partitions host-side so no partition broadcast is needed
            bias_sb = cpool.tile([P, S_LOC, O_LOC], f32)
            bsig_sb = cpool.tile([P, O_LOC], f32)
            bmu_sb = cpool.tile([P, O_LOC], f32)
            nc.sync.dma_start(bsig_sb[:], brho[:])
            nc.sync.dma_start(bmu_sb[:], bmu[:])
            nc.sync.dma_start(bias_sb[:], beps[:])
            nc.scalar.activation(bsig_sb[:], bsig_sb[:], EXP)
            nc.scalar.activation(bsig_sb[:], bsig_sb[:], LN, bias=1.0)
            for s in range(S_LOC):
                nc.vector.tensor_mul(bias_sb[:, s], bias_sb[:, s], bsig_sb[:])
                nc.vector.tensor_add(bias_sb[:, s], bias_sb[:, s], bmu_sb[:])

            H = 2               # eps DMA halves per sample
            KOH = KO // H
            for s in range(S_LOC):
                ps = [ppool.tile([P, O_LOC], f32, name=f"ps{b}") for b in range(B_TILES)]
                for h in range(H):
                    ep = epool.tile([P, KOH, O_LOC], f32, name="ep")
                    nc.sync.dma_start(ep[:], epsT[s, :, h * KOH:(h + 1) * KOH])
                    if h == 0 and s + 1 < S_LOC:
                        nc.sync.dma_start(xt_sb[:, s + 1], xT[s + 1])
                    for kk in range(KOH):
                        ko = h * KOH + kk
                        w = wpool.tile([P, O_LOC], mm_dt, name="w")
                        wt = wpool.tile([P, O_LOC], f32, name="wt")
                        nc.vector.tensor_mul(wt[:], sig_sb[:, ko], ep[:, kk])
                        nc.vector.tensor_add(w[:], wt[:], mu_sb[:, ko])
                        for b in range(B_TILES):
                            nc.tensor.matmul(
                                ps[b][:],
                                lhsT=xt_sb[:, s, ko, b * P:(b + 1) * P],
                                rhs=w[:],
                                start=(ko == 0),
                                stop=(ko == KO - 1),
                            )
                for b in range(B_TILES):
                    ot = opool.tile([P, O_LOC], f32, name="ot")
                    nc.vector.tensor_add(ot[:], ps[b][:], bias_sb[:, s])
                    nc.sync.dma_start(y[s, b], ot[:])

    nc.compile()
    _cache[key] = nc
    return nc


def kernel(**inputs):
    global LAST_RESULT
    x = np.ascontiguousarray(np.asarray(inputs["x"], dtype=np.float32))
    mu = np.asarray(inputs["weight_mu"], dtype=np.float32)
    rho = np.asarray(inputs["weight_rho"], dtype=np.float32)
    bias_mu = np.asarray(inputs["bias_mu"], dtype=np.float32)
    bias_rho = np.asarray(inputs["bias_rho"], dtype=np.float32)
    assert int(inputs["samples"]) == S

    eps_cores, eps_b = _get_eps()

    # host-side layout prep (pure transposes/slices)
    xT = np.ascontiguousarray(x.reshape(S, B, KO, P).transpose(0, 3, 2, 1))
    muT = mu.reshape(OUT, KO, P).transpose(2, 1, 0)      # [P, KO, OUT] view
    rhoT = rho.reshape(OUT, KO, P).transpose(2, 1, 0)

    in_maps = []
    for c in range(NCORES):
        g, j = divmod(c, JS)
        ssl = slice(g * S_LOC, (g + 1) * S_LOC)
        osl = slice(j * O_LOC, (j + 1) * O_LOC)
        in_maps.append({
            "xT": np.ascontiguousarray(xT[ssl]),
            "muT": np.ascontiguousarray(muT[:, :, osl]),
            "rhoT": np.ascontiguousarray(rhoT[:, :, osl]),
            "epsT": eps_cores[c],
            "bmu": np.ascontiguousarray(
                np.broadcast_to(bias_mu[osl], (P, O_LOC))),
            "brho": np.ascontiguousarray(
                np.broadcast_to(bias_rho[osl], (P, O_LOC))),
            "beps": np.ascontiguousarray(
                np.broadcast_to(eps_b[ssl, osl][None], (P, S_LOC, O_LOC))),
        })

    nc = _build_nc(MM_DT)
    from concourse import bass_utils
    res = bass_utils.run_bass_kernel_spmd(
        nc, in_maps, core_ids=list(range(NCORES)), trace=TRACE)
    LAST_RESULT = res

    out = np.empty((S, B, OUT), dtype=np.float32)
    for c in range(NCORES):
        g, j = divmod(c, JS)
        yc = res.results[c]["y"].reshape(S_LOC, B, O_LOC)
        out[g * S_LOC:(g + 1) * S_LOC, :, j * O_LOC:(j + 1) * O_LOC] = yc
    return out
